# revision 42
# baseline (speedup 1.0000x reference)
"""DiSA (directional self-attention) Bass kernel for Trainium2, 8 cores.

Factorized algorithm (no [S,S,D] intermediate): with
  w = e^{a+b} * psi(a+b),  psi(x) = exp(C*tanh(x/C) - x),  a=dep, b=head,
approximate psi by a degree-K polynomial P (weighted LS fit on the actual
a+b range).  Taylor expansion P(a+b) = sum_m b^m Q_m(a) makes the softmax
separable; e^b cancels in the ratio:
  attn_res[i,d] = sum_m b_i^m N_m(i) / sum_m b_i^m D_m(i)
  D_m(i) = suffix_{j>i}[ mask_j e^{a_j} Q_m(a_j) ]          (x rep_j for N_m)

Sharding: SOLO per batch — core c computes batch c//2 fully (pairs
duplicate work).  No collectives at all: on this runtime the first
collective pays a ~46us channel-init floor plus ~10us peer-start stagger,
which dwarfs the duplicated compute.  Host reads cores 0,2,4,6.

Mapping per core (full d=300):
  - series: DVE Horner ladders (fp16, flat [128,600] = 256 s x 300 d),
    mask folded into e^a via ACT bias; num series = den series * rep (2x tt).
  - suffix sums over j: PE matmuls, strict-upper-tri / all-ones
    stationaries; moving AP (d outer, s inner) -> d-major PSUM, K=4 ->
    [128,375] per 75-d quarter = one PSUM bank.
  - sum_m b^m X_m: DVE tensor_tensor_scan (state = b*state + X_m) over
    d-major PSUM with a b-repeat tile (0 at slot 0 of each d-block).
  - gate: pre-act accumulated in PSUM (Wf1 part early, Wf2 x attn late),
    tanh-sigmoid blend in T layout, outT [300,256] f32.
"""

import numpy as np
from math import comb as _comb

B, S, D = 4, 256, 300
C = 5.0
K = 4                  # psi polynomial degree
NS = K + 1             # slots per d-plane
WB = D * NS            # 1500 cols per c-block (slot-major series)
QD = 75                # d-planes per suffix/scan quarter
QW = QD * NS           # 375 cols per suffix psum (single PSUM bank)
NQ = D // QD           # 4 quarters

# crow packed consts: [ones(256) | b_fc(300) | b1(300) | b_f(300) | 0.5*rm(256)]
O_ONES, O_BFC, O_B1, O_BF, O_HM = 0, 256, 556, 856, 1156
CROW_W = 1412

_CACHE: dict = {}


def _q_coeffs():
    # weighted LS fit of psi on the actual a+b range (Gaussian weight
    # matching the data distribution; measured range ~[-5.6, 4.9])
    lo, hi, sig = -5.8, 5.1, 2.0
    xs = np.linspace(lo, hi, 12001)
    psi = np.exp(C * np.tanh(xs / C) - xs)
    w = np.exp(-xs ** 2 / (2 * sig ** 2)) + 1e-3
    V = np.vander(xs, K + 1, increasing=True) * w[:, None]
    c, *_ = np.linalg.lstsq(V, psi * w, rcond=None)
    return [[float(c[m + j] * _comb(m + j, m)) for j in range(K - m + 1)]
            for m in range(K + 1)]


QC = _q_coeffs()


def _chunks(total, step=128):
    return [(s, min(step, total - s)) for s in range(0, total, step)]


def _build_nc():
    import concourse.bass as bass
    import concourse.tile as tile
    from concourse import bacc, mybir

    F32 = mybir.dt.float32
    F16 = mybir.dt.float16
    AF = mybir.ActivationFunctionType
    OP = mybir.AluOpType

    nc = bacc.Bacc("TRN2", target_bir_lowering=False, debug=False, num_devices=8)

    def din(name, shape, dt=F16):
        return nc.dram_tensor(name, shape, dt, kind="ExternalInput").ap()

    inputsT_d = din("inputsT", [D, S])
    W_fcT_d = din("W_fcT", [D, D])
    W1T_d = din("W1T", [D, D])
    W2T_d = din("W2T", [D, D])
    Wf1T_d = din("Wf1T", [D, D])
    Wf2T_d = din("Wf2T", [D, D])
    crow_d = din("consts_row", [1, CROW_W])
    mats_d = din("mats", [128, 384])        # [ident | su_tri | ones]
    maskb_d = din("maskbias", [128, 2], F32)
    bcol_d = din("bfc_col", [D, 1], F32)    # b_fc per-partition column
    outT_d = nc.dram_tensor("outT", [D, S], F32, kind="ExternalOutput").ap()

    DC = _chunks(D)
    GC = [(QD * q, QD) for q in range(NQ)]  # 75-row g-quarters for gate psum

    with tile.TileContext(nc) as tc:
        with tc.tile_pool(name="persist", bufs=1) as pp:
            # ---- input DMAs: critical tensors first, spread across queues
            inT = [pp.tile([n, S], F16, tag=f"inT{i}", name=f"inT{i}") for i, (o, n) in enumerate(DC)]
            WfcT = [pp.tile([n, D], F16, tag=f"wfc{i}", name=f"wfc{i}") for i, (o, n) in enumerate(DC)]
            W1T = [pp.tile([n, D], F16, tag=f"w1{i}", name=f"w1_{i}") for i, (o, n) in enumerate(DC)]
            W2T = [pp.tile([n, D], F16, tag=f"w2{i}", name=f"w2_{i}") for i, (o, n) in enumerate(DC)]
            Wf1m = [pp.tile([QD, D], F16, tag=f"wg1{i}", name=f"wg1_{i}") for i in range(NQ)]
            Wf2m = [pp.tile([QD, D], F16, tag=f"wg2{i}", name=f"wg2_{i}") for i in range(NQ)]
            crow = pp.tile([1, CROW_W], F16)
            mats = pp.tile([128, 384], F16)
            maskb = pp.tile([128, 2], F32)
            bcol = [pp.tile([n, 1], F32, tag=f"bc{i}", name=f"bc{i}") for i, (o, n) in enumerate(DC)]

            for i, (o, n) in enumerate(DC):
                nc.sync.dma_start(inT[i][:], inputsT_d[o : o + n, :])
                nc.scalar.dma_start(WfcT[i][:], W_fcT_d[o : o + n, :])
            nc.sync.dma_start(crow[:], crow_d[:])
            nc.scalar.dma_start(maskb[:], maskb_d[:])
            for i, (o, n) in enumerate(DC):
                nc.sync.dma_start(W1T[i][:], W1T_d[o : o + n, :])
                nc.scalar.dma_start(W2T[i][:], W2T_d[o : o + n, :])
                nc.gpsimd.dma_start(bcol[i][:], bcol_d[o : o + n, :])
            nc.gpsimd.dma_start(mats[:], mats_d[:])
            for i in range(NQ):
                nc.gpsimd.dma_start(Wf1m[i][:], Wf1T_d[i * QD : (i + 1) * QD, :])
                nc.gpsimd.dma_start(Wf2m[i][:], Wf2T_d[i * QD : (i + 1) * QD, :])

            repT = [pp.tile([n, S], F16, tag=f"repT{i}", name=f"repT{i}") for i, (o, n) in enumerate(DC)]
            repTm = [pp.tile([QD, S], F16, tag=f"rtm{i}", name=f"rtm{i}") for i in range(NQ)]
            rep_nat = pp.tile([128, 2 * D], F16)
            dep_nat = pp.tile([128, 2 * D], F16)
            head_nat = pp.tile([128, 2 * D], F16)
            E_t = pp.tile([128, 2 * D], F16)
            SERd = pp.tile([128, 2 * WB], F16)   # slot-major: col = c*WB + s*D + d
            SERn = pp.tile([128, 2 * WB], F16)
            bblk = pp.tile([128, 2 * WB], F16)   # d-major: col = c*WB + d*NS + s
            scd = [pp.tile([128, WB], F16, tag=f"scd{i}", name=f"scd{i}") for i in range(2)]
            scn = [pp.tile([128, WB], F16, tag=f"scn{i}", name=f"scn{i}") for i in range(2)]
            attn_nat = pp.tile([128, 2 * D], F16)
            a75 = [pp.tile([QD, S], F16, tag=f"a75_{i}", name=f"a75_{i}") for i in range(NQ)]
            th16 = [pp.tile([QD, S], F16, tag=f"th{i}", name=f"th{i}") for i in range(NQ)]
            Mb = pp.tile([QD, S], F16)

            # ---------- phase A ----------
            with (
                tc.tile_pool(name="pa_ps", bufs=2, space="PSUM") as pa_ps,
                tc.tile_pool(name="pa_sb", bufs=2) as pa_sb,
            ):
                def elu_from_psum(ps_ap, out_ap, n, bias=None):
                    # elu(x) = min(exp(x) - 1, relu(x)); optional per-partition
                    # bias column folded into both branches.
                    w = ps_ap.shape[1]
                    ex = pa_sb.tile([n, w], F16, tag="elu_e", name="elu_e")
                    nc.scalar.activation(
                        ex[:], ps_ap, AF.Exp,
                        bias=(bias if bias is not None else 0.0),
                    )
                    rl = pa_sb.tile([n, w], F16, tag="elu_r", name="elu_r")
                    nc.scalar.activation(
                        rl[:], ps_ap, AF.Relu,
                        bias=(bias if bias is not None else 0.0),
                    )
                    nc.vector.scalar_tensor_tensor(
                        out=out_ap, in0=ex[:], scalar=-1.0, in1=rl[:],
                        op0=OP.add, op1=OP.min,
                    )

                for i, (o, n) in enumerate(DC):
                    ps = pa_ps.tile([n, S], F32, tag="paT", name="paT")
                    for k in range(3):
                        nc.tensor.matmul(
                            ps[:], WfcT[k][:, o : o + n], inT[k][:],
                            start=(k == 0), stop=(k == 2),
                        )
                    elu_from_psum(ps[:], repT[i][:], n, bias=bcol[i][:])

                # Mb = broadcast of 0.5*rep_mask row (PE outer product)
                mps = pa_ps.tile([QD, S], F32, tag="mps", name="mps")
                nc.tensor.matmul(
                    mps[:], crow[0:1, O_ONES : O_ONES + QD],
                    crow[0:1, O_HM : O_HM + S],
                    start=True, stop=True,
                )
                nc.scalar.activation(Mb[:], mps[:], AF.Copy)

                # repTm copies (sbuf->sbuf DMA, 75-row quarters of repT)
                nc.sync.dma_start(repTm[0][:], repT[0][0:QD, :])
                nc.sync.dma_start(repTm[1][0:53, :], repT[0][QD:128, :])
                nc.scalar.dma_start(repTm[1][53:QD, :], repT[1][0:22, :])
                nc.scalar.dma_start(repTm[2][:], repT[1][22:97, :])
                nc.sync.dma_start(repTm[3][0:31, :], repT[1][97:128, :])
                nc.scalar.dma_start(repTm[3][31:QD, :], repT[2][0:44, :])

                for cc in range(2):
                    so = 128 * cc
                    ps = pa_ps.tile([128, D], F32, tag="paN", name="paN")
                    for k in range(3):
                        nc.tensor.matmul(
                            ps[:], repT[k][:, so : so + 128], W1T[k][:],
                            start=(k == 0), stop=False,
                        )
                    nc.tensor.matmul(
                        ps[:], crow[0:1, O_ONES : O_ONES + 128],
                        crow[0:1, O_B1 : O_B1 + D],
                        start=False, stop=True,
                    )
                    # E = exp(dep + maskbias) straight from psum; dep copy on ACT
                    nc.scalar.activation(
                        E_t[:, cc * D : (cc + 1) * D], ps[:], AF.Exp,
                        bias=maskb[:, cc : cc + 1], scale=1.0,
                    )
                    nc.scalar.activation(
                        dep_nat[:, cc * D : (cc + 1) * D], ps[:], AF.Copy
                    )

                    ps2 = pa_ps.tile([128, D], F32, tag="paN", name="paN")
                    for k in range(3):
                        nc.tensor.matmul(
                            ps2[:], repT[k][:, so : so + 128], W2T[k][:],
                            start=(k == 0), stop=(k == 2),
                        )
                    nc.scalar.activation(
                        head_nat[:, cc * D : (cc + 1) * D], ps2[:], AF.Copy
                    )

                # rep_nat last: only needed for the (late) num-series tt
                for cc in range(2):
                    so = 128 * cc
                    ps = pa_ps.tile([128, D], F32, tag="paN", name="paN")
                    for k in range(3):
                        nc.tensor.matmul(
                            ps[:], inT[k][:, so : so + 128], WfcT[k][:],
                            start=(k == 0), stop=False,
                        )
                    # b_fc bias runs along the free (d) dim here: rank-1 add
                    nc.tensor.matmul(
                        ps[:], crow[0:1, O_ONES : O_ONES + 128],
                        crow[0:1, O_BFC : O_BFC + D],
                        start=False, stop=True,
                    )
                    elu_from_psum(ps[:], rep_nat[:, cc * D : (cc + 1) * D], 128)

            # ---------- phase B + C ----------
            with (
                tc.tile_pool(name="pb_sb", bufs=2) as pb_sb,
                tc.tile_pool(name="pb_ps", bufs=1, space="PSUM") as pb_ps,
                tc.tile_pool(name="tp_ps", bufs=1, space="PSUM") as tp_ps,
                tc.tile_pool(name="pc_ps", bufs=1, space="PSUM") as pc_ps,
                tc.tile_pool(name="pc_sb", bufs=2) as pc_sb,
            ):
                # gate Wf1 partial (early: PE stays warm during ladders)
                pcp = [pc_ps.tile([n, S], F32, tag=f"pcp{i}", name=f"pcp{i}") for i, (o, n) in enumerate(GC)]
                for i, (go, gn) in enumerate(GC):
                    for q in range(NQ):
                        nc.tensor.matmul(
                            pcp[i][:], Wf1m[q][:, go : go + gn], repTm[q][:],
                            start=(q == 0), stop=False,
                        )
                    nc.tensor.matmul(
                        pcp[i][:], crow[0:1, O_BF + go : O_BF + go + gn],
                        crow[0:1, O_ONES : O_ONES + S],
                        start=False, stop=False,
                    )

                # Horner ladders -> slot-major series (DVE).  den finals
                # first; num series = den series * rep (2x-mode tt).
                sd4 = SERd[:].rearrange("p (c s d) -> p c s d", c=2, s=NS)
                sn4 = SERn[:].rearrange("p (c s d) -> p c s d", c=2, s=NS)
                E3 = E_t[:].rearrange("p (c d) -> p c d", c=2)
                rep3 = rep_nat[:].rearrange("p (c d) -> p c d", c=2)

                for m in range(K + 1):
                    n = K - m
                    s = K - m
                    if n == 0:
                        nc.vector.tensor_scalar(
                            out=sd4[:, :, s, :], in0=E3,
                            scalar1=QC[m][0], scalar2=None, op0=OP.mult,
                        )
                        continue
                    acc = pb_sb.tile([128, 2 * D], F16, tag=f"acc{m}", name=f"acc{m}")
                    nc.vector.tensor_scalar(
                        out=acc[:], in0=dep_nat[:],
                        scalar1=QC[m][n], scalar2=None, op0=OP.mult,
                    )
                    for j in range(n - 1, 0, -1):
                        nc.vector.scalar_tensor_tensor(
                            out=acc[:], in0=acc[:], scalar=QC[m][j], in1=dep_nat[:],
                            op0=OP.add, op1=OP.mult,
                        )
                    acc3 = acc[:].rearrange("p (c d) -> p c d", c=2)
                    nc.vector.scalar_tensor_tensor(
                        out=sd4[:, :, s, :], in0=acc3, scalar=QC[m][0],
                        in1=E3, op0=OP.add, op1=OP.mult,
                    )
                for m in range(K + 1):
                    s = K - m
                    nc.vector.tensor_tensor(
                        out=sn4[:, :, s, :], in0=sd4[:, :, s, :], in1=rep3,
                        op=OP.mult,
                    )

                # b-repeat tile AFTER the ladders: its strided DVE writes
                # overlap the den suffix matmuls on the PE instead of
                # delaying the series (only scans consume it)
                bb4 = bblk[:].rearrange("p (c d s) -> p c d s", c=2, s=NS)
                h3 = head_nat[:].rearrange("p (c d) -> p c d", c=2).unsqueeze(3)
                for s in range(1, NS):
                    nc.vector.tensor_scalar(
                        out=bb4[:, :, :, s : s + 1], in0=h3,
                        scalar1=0.0, scalar2=None, op0=OP.add,
                    )
                nc.vector.memset(bb4[:, :, :, 0:1], 0.0)

                def mv_slice(ser, cc, dlo, dn):
                    # moving AP iterating (d outer, s inner) over slot-major ser
                    v = ser[:].rearrange("p (c s d) -> p c d s", c=2, s=NS)
                    return v[:, cc, dlo : dlo + dn, :]

                # suffix matmuls + scans, kind-outer so all den matmuls run
                # during the bblk build and PE stays continuously busy
                for kind in range(2):
                    ser = SERd if kind == 0 else SERn
                    outs = scd if kind == 0 else scn
                    for q in range(NQ):
                        dlo = QD * q
                        p0 = pb_ps.tile([128, QW], F32, tag="sx0", name="sx0")
                        p1 = pb_ps.tile([128, QW], F32, tag="sx1", name="sx1")
                        nc.tensor.matmul(
                            p0[:], mats[:, 128:256], mv_slice(ser, 0, dlo, QD),
                            start=True, stop=False,
                        )
                        nc.tensor.matmul(
                            p1[:], mats[:, 128:256], mv_slice(ser, 1, dlo, QD),
                            start=True, stop=True,
                        )
                        nc.tensor.matmul(
                            p0[:], mats[:, 256:384], mv_slice(ser, 1, dlo, QD),
                            start=False, stop=True,
                        )
                        for cc in range(2):
                            nc.vector.tensor_tensor_scan(
                                out=outs[cc][:, q * QW : (q + 1) * QW],
                                data0=bblk[:, cc * WB + q * QW : cc * WB + (q + 1) * QW],
                                data1=(p0 if cc == 0 else p1)[:],
                                initial=0.0, op0=OP.mult, op1=OP.add,
                            )

                # divides + transposes per quarter
                for q in range(NQ):
                    dlo = QD * q
                    for cc in range(2):
                        dv = scd[cc][:, q * QW : (q + 1) * QW].rearrange(
                            "p (d s) -> p d s", s=NS)[:, :, K : K + 1]
                        nv = scn[cc][:, q * QW : (q + 1) * QW].rearrange(
                            "p (d s) -> p d s", s=NS)[:, :, K : K + 1]
                        den0 = pb_sb.tile([128, QD], F32, tag="den0", name="den0")
                        nc.vector.scalar_tensor_tensor(
                            out=den0[:].unsqueeze(2), in0=dv, scalar=0.0, in1=dv,
                            op0=OP.is_equal, op1=OP.add,
                        )
                        rcp = pb_sb.tile([128, QD], F32, tag="rcp", name="rcp")
                        nc.vector.reciprocal(out=rcp[:], in_=den0[:])
                        nc.vector.tensor_tensor(
                            out=attn_nat[:, cc * D + dlo : cc * D + dlo + QD].unsqueeze(2),
                            in0=nv, in1=rcp[:].unsqueeze(2), op=OP.mult,
                        )
                    for cc in range(2):
                        t75 = tp_ps.tile([QD, 128], F16, tag="t75", name="t75")
                        nc.tensor.transpose(
                            t75[:], attn_nat[:, cc * D + dlo : cc * D + dlo + QD],
                            mats[:, 0:128],
                        )
                        nc.scalar.activation(
                            a75[q][:, cc * 128 : (cc + 1) * 128], t75[:], AF.Copy
                        )
                    # Wf2 gate accumulation interleaved per quarter
                    for i, (go, gn) in enumerate(GC):
                        nc.tensor.matmul(
                            pcp[i][:], Wf2m[q][:, go : go + gn], a75[q][:],
                            start=False, stop=(q == NQ - 1),
                        )

                for i in range(NQ):
                    nc.scalar.activation(th16[i][:], pcp[i][:], AF.Tanh, scale=0.5)

                for q in range(NQ):
                    diff = pc_sb.tile([QD, S], F16, tag="diff", name="diff")
                    nc.vector.tensor_tensor(
                        out=diff[:], in0=repTm[q][:], in1=a75[q][:], op=OP.subtract
                    )
                    summ = pc_sb.tile([QD, S], F16, tag="summ", name="summ")
                    nc.vector.tensor_tensor(
                        out=summ[:], in0=repTm[q][:], in1=a75[q][:], op=OP.add
                    )
                    nc.vector.tensor_tensor(
                        out=diff[:], in0=th16[q][:], in1=diff[:], op=OP.mult
                    )
                    nc.vector.tensor_tensor(
                        out=summ[:], in0=summ[:], in1=diff[:], op=OP.add
                    )
                    outt = pc_sb.tile([QD, S], F32, tag="outt", name="outt")
                    nc.vector.tensor_tensor(
                        out=outt[:], in0=summ[:], in1=Mb[:], op=OP.mult
                    )
                    eng_o = nc.scalar if q % 2 == 1 else nc.sync
                    eng_o.dma_start(outT_d[q * QD : (q + 1) * QD, :], outt[:])

    nc.compile()
    return nc


def _host_prep(inputs, rep_mask, W_fc, b_fc, W1, W2, b1, W_f1, W_f2, b_f):
    f = np.float32
    h = np.float16
    su = (np.arange(128)[:, None] > np.arange(128)[None, :]).astype(h)
    mats = np.concatenate(
        [np.eye(128, dtype=h), su, np.ones((128, 128), dtype=h)], axis=1
    )
    in_maps = []
    for c in range(8):
        b = c // 2
        rm = rep_mask[b].astype(f)
        maskbias = np.stack(
            [(rm[0:128] - 1.0) * 30000.0, (rm[128:256] - 1.0) * 30000.0], axis=1
        ).astype(f)
        crow = np.zeros(CROW_W, dtype=h)
        crow[O_ONES : O_ONES + S] = 1.0
        crow[O_BFC : O_BFC + D] = b_fc.astype(h)
        crow[O_B1 : O_B1 + D] = b1.astype(h)
        crow[O_BF : O_BF + D] = b_f.astype(h)
        crow[O_HM : O_HM + S] = (0.5 * rm).astype(h)
        mp = {
            "inputsT": np.ascontiguousarray(inputs[b].T).astype(h),
            "W_fcT": np.ascontiguousarray(W_fc.T).astype(h),
            "W1T": np.ascontiguousarray(W1.T).astype(h),
            "W2T": np.ascontiguousarray(W2.T).astype(h),
            "Wf1T": np.ascontiguousarray(W_f1.T).astype(h),
            "Wf2T": np.ascontiguousarray(W_f2.T).astype(h),
            "consts_row": crow.reshape(1, CROW_W),
            "mats": mats,
            "maskbias": maskbias,
            "bfc_col": b_fc.astype(f).reshape(D, 1),
        }
        in_maps.append(mp)
    return in_maps


def _assemble(results):
    out = np.empty((B, S, D), dtype=np.float32)
    for b in range(B):
        out[b] = results[2 * b]["outT"].T
    return out


def kernel(**inputs):
    from concourse.bass_utils import run_bass_kernel_spmd

    if "nc" not in _CACHE:
        _CACHE["nc"] = _build_nc()
    nc = _CACHE["nc"]

    in_maps = _host_prep(**inputs)
    res = run_bass_kernel_spmd(nc, in_maps, list(range(8)))
    return _assemble(res.results)


# revision 48
# speedup vs baseline: 1.1938x; 1.1938x over previous
"""DiSA (directional self-attention) Bass kernel for Trainium2, 8 cores.

Factorized algorithm (no [S,S,D] intermediate): with
  w = e^{a+b} * psi(a+b),  psi(x) = exp(C*tanh(x/C) - x),  a=dep, b=head,
approximate psi by a degree-K polynomial P (weighted LS fit on the actual
a+b range).  Taylor expansion P(a+b) = sum_m b^m Q_m(a) makes the softmax
separable; e^b cancels in the ratio:
  attn_res[i,d] = sum_m b_i^m N_m(i) / sum_m b_i^m D_m(i)
  D_m(i) = suffix_{j>i}[ mask_j e^{a_j} Q_m(a_j) ]          (x rep_j for N_m)

Sharding: SOLO per batch — core c computes batch c//2 fully (pairs
duplicate work).  No collectives at all: on this runtime the first
collective pays a ~46us channel-init floor plus ~10us peer-start stagger,
which dwarfs the duplicated compute.  Host reads cores 0,2,4,6.

Mapping per core (full d=300):
  - series: DVE Horner ladders (fp16, flat [128,600] = 256 s x 300 d),
    mask folded into e^a via ACT bias; num series = den series * rep (2x tt).
  - suffix sums over j: PE matmuls, strict-upper-tri / all-ones
    stationaries; moving AP (d outer, s inner) -> d-major PSUM, K=4 ->
    [128,375] per 75-d quarter = one PSUM bank.
  - sum_m b^m X_m: DVE tensor_tensor_scan (state = b*state + X_m) over
    d-major PSUM with a b-repeat tile (0 at slot 0 of each d-block).
  - gate: pre-act accumulated in PSUM (Wf1 part early, Wf2 x attn late),
    tanh-sigmoid blend in T layout, outT [300,256] f32.
"""

import numpy as np
from math import comb as _comb

B, S, D = 4, 256, 300
C = 5.0
K = 4                  # psi polynomial degree
NS = K + 1             # slots per d-plane
WB = D * NS            # 1500 cols per c-block (slot-major series)
QD = 75                # d-planes per suffix/scan quarter
QW = QD * NS           # 375 cols per suffix psum (single PSUM bank)
NQ = D // QD           # 4 quarters

# crow packed consts: [ones(256) | b_fc(300) | b1(300) | b_f(300) | 0.5*rm(256)]
O_ONES, O_BFC, O_B1, O_BF, O_HM = 0, 256, 556, 856, 1156
CROW_W = 1412

_CACHE: dict = {}


def _q_coeffs():
    # weighted LS fit of psi on the actual a+b range (Gaussian weight
    # matching the data distribution; measured range ~[-5.6, 4.9])
    lo, hi, sig = -5.8, 5.1, 2.0
    xs = np.linspace(lo, hi, 12001)
    psi = np.exp(C * np.tanh(xs / C) - xs)
    w = np.exp(-xs ** 2 / (2 * sig ** 2)) + 1e-3
    V = np.vander(xs, K + 1, increasing=True) * w[:, None]
    c, *_ = np.linalg.lstsq(V, psi * w, rcond=None)
    return [[float(c[m + j] * _comb(m + j, m)) for j in range(K - m + 1)]
            for m in range(K + 1)]


QC = _q_coeffs()


def _chunks(total, step=128):
    return [(s, min(step, total - s)) for s in range(0, total, step)]


def _build_nc():
    import concourse.bass as bass
    import concourse.tile as tile
    from concourse import bacc, mybir

    F32 = mybir.dt.float32
    F16 = mybir.dt.float16
    AF = mybir.ActivationFunctionType
    OP = mybir.AluOpType

    nc = bacc.Bacc("TRN2", target_bir_lowering=False, debug=False, num_devices=8)

    def din(name, shape, dt=F16):
        return nc.dram_tensor(name, shape, dt, kind="ExternalInput").ap()

    inputsT_d = din("inputsT", [D, S])
    W_fcT_d = din("W_fcT", [D, D])
    W1T_d = din("W1T", [D, D])
    W2T_d = din("W2T", [D, D])
    Wf1T_d = din("Wf1T", [D, D])
    Wf2T_d = din("Wf2T", [D, D])
    crow_d = din("consts_row", [1, CROW_W])
    mats_d = din("mats", [128, 384])        # [ident | su_tri | ones]
    maskb_d = din("maskbias", [128, 2], F32)
    bcol_d = din("bfc_col", [D, 1], F32)    # b_fc per-partition column
    outT_d = nc.dram_tensor("outT", [D, S], F32, kind="ExternalOutput").ap()

    DC = _chunks(D)
    GC = [(QD * q, QD) for q in range(NQ)]  # 75-row g-quarters for gate psum

    with tile.TileContext(nc) as tc:
        with tc.tile_pool(name="persist", bufs=1) as pp:
            # ---- input DMAs: critical tensors first, spread across queues
            inT = [pp.tile([n, S], F16, tag=f"inT{i}", name=f"inT{i}") for i, (o, n) in enumerate(DC)]
            WfcT = [pp.tile([n, D], F16, tag=f"wfc{i}", name=f"wfc{i}") for i, (o, n) in enumerate(DC)]
            W1T = [pp.tile([n, D], F16, tag=f"w1{i}", name=f"w1_{i}") for i, (o, n) in enumerate(DC)]
            W2T = [pp.tile([n, D], F16, tag=f"w2{i}", name=f"w2_{i}") for i, (o, n) in enumerate(DC)]
            Wf1m = [pp.tile([QD, D], F16, tag=f"wg1{i}", name=f"wg1_{i}") for i in range(NQ)]
            Wf2m = [pp.tile([QD, D], F16, tag=f"wg2{i}", name=f"wg2_{i}") for i in range(NQ)]
            crow = pp.tile([1, CROW_W], F16)
            mats = pp.tile([128, 384], F16)
            maskb = pp.tile([128, 2], F32)
            bcol = [pp.tile([n, 1], F32, tag=f"bc{i}", name=f"bc{i}") for i, (o, n) in enumerate(DC)]

            for i, (o, n) in enumerate(DC):
                nc.sync.dma_start(inT[i][:], inputsT_d[o : o + n, :])
                nc.scalar.dma_start(WfcT[i][:], W_fcT_d[o : o + n, :])
            nc.sync.dma_start(crow[:], crow_d[:])
            nc.scalar.dma_start(maskb[:], maskb_d[:])
            for i, (o, n) in enumerate(DC):
                nc.sync.dma_start(W1T[i][:], W1T_d[o : o + n, :])
                nc.scalar.dma_start(W2T[i][:], W2T_d[o : o + n, :])
                nc.gpsimd.dma_start(bcol[i][:], bcol_d[o : o + n, :])
            nc.gpsimd.dma_start(mats[:], mats_d[:])
            for i in range(NQ):
                nc.gpsimd.dma_start(Wf1m[i][:], Wf1T_d[i * QD : (i + 1) * QD, :])
                nc.gpsimd.dma_start(Wf2m[i][:], Wf2T_d[i * QD : (i + 1) * QD, :])

            repT = [pp.tile([n, S], F16, tag=f"repT{i}", name=f"repT{i}") for i, (o, n) in enumerate(DC)]
            repTm = [pp.tile([QD, S], F16, tag=f"rtm{i}", name=f"rtm{i}") for i in range(NQ)]
            rep_nat = pp.tile([128, 2 * D], F16)
            dep_nat = pp.tile([128, 2 * D], F16)
            head_nat = pp.tile([128, 2 * D], F16)
            E_t = pp.tile([128, 2 * D], F16)
            SERd = pp.tile([128, 2 * WB], F16)   # slot-major: col = c*WB + s*D + d
            SERn = pp.tile([128, 2 * WB], F16)
            bblk = pp.tile([128, 2 * WB], F16)   # d-major: col = c*WB + d*NS + s
            scd = [pp.tile([128, WB], F16, tag=f"scd{i}", name=f"scd{i}") for i in range(2)]
            scn = [pp.tile([128, WB], F16, tag=f"scn{i}", name=f"scn{i}") for i in range(2)]
            attn_nat = pp.tile([128, 2 * D], F16)
            a75 = [pp.tile([QD, S], F16, tag=f"a75_{i}", name=f"a75_{i}") for i in range(NQ)]
            th16 = [pp.tile([QD, S], F16, tag=f"th{i}", name=f"th{i}") for i in range(NQ)]
            Mb = pp.tile([QD, S], F16)

            # ---------- phase A ----------
            with (
                tc.tile_pool(name="pa_ps", bufs=2, space="PSUM") as pa_ps,
                tc.tile_pool(name="pa_sb", bufs=2) as pa_sb,
            ):
                def elu_from_psum(ps_ap, out_ap, n, bias=None):
                    # elu(x) = min(exp(x) - 1, relu(x)); optional per-partition
                    # bias column folded into both branches.
                    w = ps_ap.shape[1]
                    ex = pa_sb.tile([n, w], F16, tag="elu_e", name="elu_e")
                    nc.scalar.activation(
                        ex[:], ps_ap, AF.Exp,
                        bias=(bias if bias is not None else 0.0),
                    )
                    rl = pa_sb.tile([n, w], F16, tag="elu_r", name="elu_r")
                    nc.scalar.activation(
                        rl[:], ps_ap, AF.Relu,
                        bias=(bias if bias is not None else 0.0),
                    )
                    nc.vector.scalar_tensor_tensor(
                        out=out_ap, in0=ex[:], scalar=-1.0, in1=rl[:],
                        op0=OP.add, op1=OP.min,
                    )

                for i, (o, n) in enumerate(DC):
                    ps = pa_ps.tile([n, S], F32, tag="paT", name="paT")
                    for k in range(3):
                        nc.tensor.matmul(
                            ps[:], WfcT[k][:, o : o + n], inT[k][:],
                            start=(k == 0), stop=(k == 2),
                        )
                    elu_from_psum(ps[:], repT[i][:], n, bias=bcol[i][:])

                # Mb = broadcast of 0.5*rep_mask row (PE outer product)
                mps = pa_ps.tile([QD, S], F32, tag="mps", name="mps")
                nc.tensor.matmul(
                    mps[:], crow[0:1, O_ONES : O_ONES + QD],
                    crow[0:1, O_HM : O_HM + S],
                    start=True, stop=True,
                )
                nc.scalar.activation(Mb[:], mps[:], AF.Copy)

                # repTm copies (sbuf->sbuf DMA, 75-row quarters of repT)
                nc.sync.dma_start(repTm[0][:], repT[0][0:QD, :])
                nc.sync.dma_start(repTm[1][0:53, :], repT[0][QD:128, :])
                nc.scalar.dma_start(repTm[1][53:QD, :], repT[1][0:22, :])
                nc.scalar.dma_start(repTm[2][:], repT[1][22:97, :])
                nc.sync.dma_start(repTm[3][0:31, :], repT[1][97:128, :])
                nc.scalar.dma_start(repTm[3][31:QD, :], repT[2][0:44, :])

                for cc in range(2):
                    so = 128 * cc
                    ps = pa_ps.tile([128, D], F32, tag="paN", name="paN")
                    for k in range(3):
                        nc.tensor.matmul(
                            ps[:], repT[k][:, so : so + 128], W1T[k][:],
                            start=(k == 0), stop=False,
                        )
                    nc.tensor.matmul(
                        ps[:], crow[0:1, O_ONES : O_ONES + 128],
                        crow[0:1, O_B1 : O_B1 + D],
                        start=False, stop=True,
                    )
                    # E = exp(dep + maskbias) straight from psum; dep copy on ACT
                    nc.scalar.activation(
                        E_t[:, cc * D : (cc + 1) * D], ps[:], AF.Exp,
                        bias=maskb[:, cc : cc + 1], scale=1.0,
                    )
                    nc.scalar.activation(
                        dep_nat[:, cc * D : (cc + 1) * D], ps[:], AF.Copy
                    )

                    ps2 = pa_ps.tile([128, D], F32, tag="paN", name="paN")
                    for k in range(3):
                        nc.tensor.matmul(
                            ps2[:], repT[k][:, so : so + 128], W2T[k][:],
                            start=(k == 0), stop=(k == 2),
                        )
                    nc.scalar.activation(
                        head_nat[:, cc * D : (cc + 1) * D], ps2[:], AF.Copy
                    )

                # rep_nat last: only needed for the (late) num-series tt
                for cc in range(2):
                    so = 128 * cc
                    ps = pa_ps.tile([128, D], F32, tag="paN", name="paN")
                    for k in range(3):
                        nc.tensor.matmul(
                            ps[:], inT[k][:, so : so + 128], WfcT[k][:],
                            start=(k == 0), stop=False,
                        )
                    # b_fc bias runs along the free (d) dim here: rank-1 add
                    nc.tensor.matmul(
                        ps[:], crow[0:1, O_ONES : O_ONES + 128],
                        crow[0:1, O_BFC : O_BFC + D],
                        start=False, stop=True,
                    )
                    elu_from_psum(ps[:], rep_nat[:, cc * D : (cc + 1) * D], 128)

            # ---------- phase B + C ----------
            with (
                tc.tile_pool(name="pb_sb", bufs=2) as pb_sb,
                tc.tile_pool(name="pb_ps", bufs=1, space="PSUM") as pb_ps,
                tc.tile_pool(name="tp_ps", bufs=1, space="PSUM") as tp_ps,
                tc.tile_pool(name="pc_ps", bufs=1, space="PSUM") as pc_ps,
                tc.tile_pool(name="pc_sb", bufs=2) as pc_sb,
            ):
                # gate Wf1 partial (early: PE stays warm during ladders)
                # gate psum: one [75, 256] tile per g-quarter (a packed
                # 2-quarters-per-bank layout breaks matmul accumulation —
                # two open accumulation groups in one tile corrupt results)
                pcp = [pc_ps.tile([QD, S], F32, tag=f"pcp{i}", name=f"pcp{i}") for i in range(NQ)]

                def pcp_sl(i):
                    return pcp[i][:]

                for i, (go, gn) in enumerate(GC):
                    for q in range(NQ):
                        nc.tensor.matmul(
                            pcp_sl(i), Wf1m[q][:, go : go + gn], repTm[q][:],
                            start=(q == 0), stop=False,
                        )
                    nc.tensor.matmul(
                        pcp_sl(i), crow[0:1, O_BF + go : O_BF + go + gn],
                        crow[0:1, O_ONES : O_ONES + S],
                        start=False, stop=False,
                    )

                # Horner ladders -> slot-major series (DVE).  den finals
                # first; num series = den series * rep (2x-mode tt).
                sd4 = SERd[:].rearrange("p (c s d) -> p c s d", c=2, s=NS)
                sn4 = SERn[:].rearrange("p (c s d) -> p c s d", c=2, s=NS)
                E3 = E_t[:].rearrange("p (c d) -> p c d", c=2)
                rep3 = rep_nat[:].rearrange("p (c d) -> p c d", c=2)

                for m in range(K + 1):
                    n = K - m
                    s = K - m
                    if n == 0:
                        nc.vector.tensor_scalar(
                            out=sd4[:, :, s, :], in0=E3,
                            scalar1=QC[m][0], scalar2=None, op0=OP.mult,
                        )
                        continue
                    acc = pb_sb.tile([128, 2 * D], F16, tag=f"acc{m}", name=f"acc{m}")
                    nc.vector.tensor_scalar(
                        out=acc[:], in0=dep_nat[:],
                        scalar1=QC[m][n], scalar2=None, op0=OP.mult,
                    )
                    for j in range(n - 1, 0, -1):
                        nc.vector.scalar_tensor_tensor(
                            out=acc[:], in0=acc[:], scalar=QC[m][j], in1=dep_nat[:],
                            op0=OP.add, op1=OP.mult,
                        )
                    acc3 = acc[:].rearrange("p (c d) -> p c d", c=2)
                    nc.vector.scalar_tensor_tensor(
                        out=sd4[:, :, s, :], in0=acc3, scalar=QC[m][0],
                        in1=E3, op0=OP.add, op1=OP.mult,
                    )
                for m in range(K + 1):
                    s = K - m
                    nc.vector.tensor_tensor(
                        out=sn4[:, :, s, :], in0=sd4[:, :, s, :], in1=rep3,
                        op=OP.mult,
                    )

                # b-repeat tile AFTER the ladders: its strided DVE writes
                # overlap the den suffix matmuls on the PE instead of
                # delaying the series (only scans consume it)
                bb4 = bblk[:].rearrange("p (c d s) -> p c d s", c=2, s=NS)
                h3 = head_nat[:].rearrange("p (c d) -> p c d", c=2).unsqueeze(3)
                for s in range(1, NS):
                    nc.vector.tensor_scalar(
                        out=bb4[:, :, :, s : s + 1], in0=h3,
                        scalar1=0.0, scalar2=None, op0=OP.add,
                    )
                nc.vector.memset(bb4[:, :, :, 0:1], 0.0)

                def mv_slice(ser, cc, dlo, dn):
                    # moving AP iterating (d outer, s inner) over slot-major ser
                    v = ser[:].rearrange("p (c s d) -> p c d s", c=2, s=NS)
                    return v[:, cc, dlo : dlo + dn, :]

                # suffix matmuls + scans, kind-outer so all den matmuls run
                # during the bblk build and PE stays continuously busy
                for kind in range(2):
                    ser = SERd if kind == 0 else SERn
                    outs = scd if kind == 0 else scn
                    for q in range(NQ):
                        dlo = QD * q
                        p0 = pb_ps.tile([128, QW], F32, tag="sx0", name="sx0")
                        p1 = pb_ps.tile([128, QW], F32, tag="sx1", name="sx1")
                        nc.tensor.matmul(
                            p0[:], mats[:, 128:256], mv_slice(ser, 0, dlo, QD),
                            start=True, stop=False,
                        )
                        nc.tensor.matmul(
                            p0[:], mats[:, 256:384], mv_slice(ser, 1, dlo, QD),
                            start=False, stop=True,
                        )
                        nc.tensor.matmul(
                            p1[:], mats[:, 128:256], mv_slice(ser, 1, dlo, QD),
                            start=True, stop=True,
                        )
                        for cc in range(2):
                            nc.vector.tensor_tensor_scan(
                                out=outs[cc][:, q * QW : (q + 1) * QW],
                                data0=bblk[:, cc * WB + q * QW : cc * WB + (q + 1) * QW],
                                data1=(p0 if cc == 0 else p1)[:],
                                initial=0.0, op0=OP.mult, op1=OP.add,
                            )

                # divides + transposes per quarter
                for q in range(NQ):
                    dlo = QD * q
                    for cc in range(2):
                        dv = scd[cc][:, q * QW : (q + 1) * QW].rearrange(
                            "p (d s) -> p d s", s=NS)[:, :, K : K + 1]
                        nv = scn[cc][:, q * QW : (q + 1) * QW].rearrange(
                            "p (d s) -> p d s", s=NS)[:, :, K : K + 1]
                        den0 = pb_sb.tile([128, QD], F32, tag="den0", name="den0")
                        nc.vector.scalar_tensor_tensor(
                            out=den0[:].unsqueeze(2), in0=dv, scalar=0.0, in1=dv,
                            op0=OP.is_equal, op1=OP.add,
                        )
                        rcp = pb_sb.tile([128, QD], F32, tag="rcp", name="rcp")
                        nc.vector.reciprocal(out=rcp[:], in_=den0[:])
                        nc.vector.tensor_tensor(
                            out=attn_nat[:, cc * D + dlo : cc * D + dlo + QD].unsqueeze(2),
                            in0=nv, in1=rcp[:].unsqueeze(2), op=OP.mult,
                        )
                    for cc in range(2):
                        t75 = tp_ps.tile([QD, 128], F16, tag="t75", name="t75")
                        nc.tensor.transpose(
                            t75[:], attn_nat[:, cc * D + dlo : cc * D + dlo + QD],
                            mats[:, 0:128],
                        )
                        nc.scalar.activation(
                            a75[q][:, cc * 128 : (cc + 1) * 128], t75[:], AF.Copy
                        )
                    # Wf2 gate accumulation interleaved per quarter
                    for i, (go, gn) in enumerate(GC):
                        nc.tensor.matmul(
                            pcp_sl(i), Wf2m[q][:, go : go + gn], a75[q][:],
                            start=False, stop=(q == NQ - 1),
                        )

                for i in range(NQ):
                    nc.scalar.activation(th16[i][:], pcp_sl(i), AF.Tanh, scale=0.5)

                for q in range(NQ):
                    diff = pc_sb.tile([QD, S], F16, tag="diff", name="diff")
                    nc.vector.tensor_tensor(
                        out=diff[:], in0=repTm[q][:], in1=a75[q][:], op=OP.subtract
                    )
                    summ = pc_sb.tile([QD, S], F16, tag="summ", name="summ")
                    nc.vector.tensor_tensor(
                        out=summ[:], in0=repTm[q][:], in1=a75[q][:], op=OP.add
                    )
                    nc.vector.tensor_tensor(
                        out=diff[:], in0=th16[q][:], in1=diff[:], op=OP.mult
                    )
                    nc.vector.tensor_tensor(
                        out=summ[:], in0=summ[:], in1=diff[:], op=OP.add
                    )
                    outt = pc_sb.tile([QD, S], F32, tag="outt", name="outt")
                    nc.vector.tensor_tensor(
                        out=outt[:], in0=summ[:], in1=Mb[:], op=OP.mult
                    )
                    eng_o = nc.scalar if q % 2 == 1 else nc.sync
                    eng_o.dma_start(outT_d[q * QD : (q + 1) * QD, :], outt[:])

    nc.compile()
    return nc


def _host_prep(inputs, rep_mask, W_fc, b_fc, W1, W2, b1, W_f1, W_f2, b_f):
    f = np.float32
    h = np.float16
    su = (np.arange(128)[:, None] > np.arange(128)[None, :]).astype(h)
    mats = np.concatenate(
        [np.eye(128, dtype=h), su, np.ones((128, 128), dtype=h)], axis=1
    )
    in_maps = []
    for c in range(8):
        b = c // 2
        rm = rep_mask[b].astype(f)
        maskbias = np.stack(
            [(rm[0:128] - 1.0) * 30000.0, (rm[128:256] - 1.0) * 30000.0], axis=1
        ).astype(f)
        crow = np.zeros(CROW_W, dtype=h)
        crow[O_ONES : O_ONES + S] = 1.0
        crow[O_BFC : O_BFC + D] = b_fc.astype(h)
        crow[O_B1 : O_B1 + D] = b1.astype(h)
        crow[O_BF : O_BF + D] = b_f.astype(h)
        crow[O_HM : O_HM + S] = (0.5 * rm).astype(h)
        mp = {
            "inputsT": np.ascontiguousarray(inputs[b].T).astype(h),
            "W_fcT": np.ascontiguousarray(W_fc.T).astype(h),
            "W1T": np.ascontiguousarray(W1.T).astype(h),
            "W2T": np.ascontiguousarray(W2.T).astype(h),
            "Wf1T": np.ascontiguousarray(W_f1.T).astype(h),
            "Wf2T": np.ascontiguousarray(W_f2.T).astype(h),
            "consts_row": crow.reshape(1, CROW_W),
            "mats": mats,
            "maskbias": maskbias,
            "bfc_col": b_fc.astype(f).reshape(D, 1),
        }
        in_maps.append(mp)
    return in_maps


def _assemble(results):
    out = np.empty((B, S, D), dtype=np.float32)
    for b in range(B):
        out[b] = results[2 * b]["outT"].T
    return out


def kernel(**inputs):
    from concourse.bass_utils import run_bass_kernel_spmd

    if "nc" not in _CACHE:
        _CACHE["nc"] = _build_nc()
    nc = _CACHE["nc"]

    in_maps = _host_prep(**inputs)
    res = run_bass_kernel_spmd(nc, in_maps, list(range(8)))
    return _assemble(res.results)


# revision 51
# speedup vs baseline: 1.2202x; 1.0221x over previous
"""DiSA (directional self-attention) Bass kernel for Trainium2, 8 cores.

Factorized algorithm (no [S,S,D] intermediate): with
  w = e^{a+b} * psi(a+b),  psi(x) = exp(C*tanh(x/C) - x),  a=dep, b=head,
approximate psi by a degree-K polynomial P (weighted LS fit on the actual
a+b range).  Taylor expansion P(a+b) = sum_m b^m Q_m(a) makes the softmax
separable; e^b cancels in the ratio:
  attn_res[i,d] = sum_m b_i^m N_m(i) / sum_m b_i^m D_m(i)
  D_m(i) = suffix_{j>i}[ mask_j e^{a_j} Q_m(a_j) ]          (x rep_j for N_m)

Sharding: SOLO per batch — core c computes batch c//2 fully (pairs
duplicate work).  No collectives at all: on this runtime the first
collective pays a ~46us channel-init floor plus ~10us peer-start stagger,
which dwarfs the duplicated compute.  Host reads cores 0,2,4,6.

Mapping per core (full d=300):
  - series: DVE Horner ladders (fp16, flat [128,600] = 256 s x 300 d),
    mask folded into e^a via ACT bias; num series = den series * rep (2x tt).
  - suffix sums over j: PE matmuls, strict-upper-tri / all-ones
    stationaries; moving AP (d outer, s inner) -> d-major PSUM, K=4 ->
    [128,375] per 75-d quarter = one PSUM bank.
  - sum_m b^m X_m: DVE tensor_tensor_scan (state = b*state + X_m) over
    d-major PSUM with a b-repeat tile (0 at slot 0 of each d-block).
  - gate: pre-act accumulated in PSUM (Wf1 part early, Wf2 x attn late),
    tanh-sigmoid blend in T layout, outT [300,256] f32.
"""

import numpy as np
from math import comb as _comb

B, S, D = 4, 256, 300
C = 5.0
K = 4                  # psi polynomial degree
NS = K + 1             # slots per d-plane
WB = D * NS            # 1500 cols per c-block (slot-major series)
QD = 75                # d-planes per suffix/scan quarter
QW = QD * NS           # 375 cols per suffix psum (single PSUM bank)
NQ = D // QD           # 4 quarters

# crow packed consts: [ones(256) | b_fc(300) | b1(300) | b_f(300) | 0.5*rm(256)]
O_ONES, O_BFC, O_B1, O_BF, O_HM = 0, 256, 556, 856, 1156
CROW_W = 1412

_CACHE: dict = {}


def _q_coeffs():
    # weighted LS fit of psi on the actual a+b range (Gaussian weight
    # matching the data distribution; measured range ~[-5.6, 4.9])
    lo, hi, sig = -5.8, 5.1, 2.0
    xs = np.linspace(lo, hi, 12001)
    psi = np.exp(C * np.tanh(xs / C) - xs)
    w = np.exp(-xs ** 2 / (2 * sig ** 2)) + 1e-3
    V = np.vander(xs, K + 1, increasing=True) * w[:, None]
    c, *_ = np.linalg.lstsq(V, psi * w, rcond=None)
    return [[float(c[m + j] * _comb(m + j, m)) for j in range(K - m + 1)]
            for m in range(K + 1)]


QC = _q_coeffs()


def _chunks(total, step=128):
    return [(s, min(step, total - s)) for s in range(0, total, step)]


def _build_nc():
    import concourse.bass as bass
    import concourse.tile as tile
    from concourse import bacc, mybir

    F32 = mybir.dt.float32
    F16 = mybir.dt.float16
    AF = mybir.ActivationFunctionType
    OP = mybir.AluOpType

    nc = bacc.Bacc("TRN2", target_bir_lowering=False, debug=False, num_devices=8)

    def din(name, shape, dt=F16):
        return nc.dram_tensor(name, shape, dt, kind="ExternalInput").ap()

    inputsT_d = din("inputsT", [D, S])
    W_fcT_d = din("W_fcT", [D, D])
    W1T_d = din("W1T", [D, D])
    W2T_d = din("W2T", [D, D])
    Wf1T_d = din("Wf1T", [D, D])
    Wf2T_d = din("Wf2T", [D, D])
    crow_d = din("consts_row", [1, CROW_W])
    mats_d = din("mats", [128, 384])        # [ident | su_tri | ones]
    maskb_d = din("maskbias", [128, 2], F32)
    bcol_d = din("bfc_col", [D, 1], F32)    # b_fc per-partition column
    outT_d = nc.dram_tensor("outT", [D, S], F32, kind="ExternalOutput").ap()

    DC = _chunks(D)
    GC = [(QD * q, QD) for q in range(NQ)]  # 75-row g-quarters for gate psum

    with tile.TileContext(nc) as tc:
        with tc.tile_pool(name="persist", bufs=1) as pp:
            # ---- input DMAs: critical tensors first, spread across queues
            inT = [pp.tile([n, S], F16, tag=f"inT{i}", name=f"inT{i}") for i, (o, n) in enumerate(DC)]
            WfcT = [pp.tile([n, D], F16, tag=f"wfc{i}", name=f"wfc{i}") for i, (o, n) in enumerate(DC)]
            W1T = [pp.tile([n, D], F16, tag=f"w1{i}", name=f"w1_{i}") for i, (o, n) in enumerate(DC)]
            W2T = [pp.tile([n, D], F16, tag=f"w2{i}", name=f"w2_{i}") for i, (o, n) in enumerate(DC)]
            Wf1m = [pp.tile([QD, D], F16, tag=f"wg1{i}", name=f"wg1_{i}") for i in range(NQ)]
            Wf2m = [pp.tile([QD, D], F16, tag=f"wg2{i}", name=f"wg2_{i}") for i in range(NQ)]
            crow = pp.tile([1, CROW_W], F16)
            mats = pp.tile([128, 384], F16)
            maskb = pp.tile([128, 2], F32)
            bcol = [pp.tile([n, 1], F32, tag=f"bc{i}", name=f"bc{i}") for i, (o, n) in enumerate(DC)]

            for i, (o, n) in enumerate(DC):
                nc.sync.dma_start(inT[i][:], inputsT_d[o : o + n, :])
                nc.scalar.dma_start(WfcT[i][:], W_fcT_d[o : o + n, :])
            nc.sync.dma_start(crow[:], crow_d[:])
            nc.scalar.dma_start(maskb[:], maskb_d[:])
            for i, (o, n) in enumerate(DC):
                nc.sync.dma_start(W1T[i][:], W1T_d[o : o + n, :])
                nc.scalar.dma_start(W2T[i][:], W2T_d[o : o + n, :])
                nc.gpsimd.dma_start(bcol[i][:], bcol_d[o : o + n, :])
            nc.gpsimd.dma_start(mats[:], mats_d[:])
            for i in range(NQ):
                nc.gpsimd.dma_start(Wf1m[i][:], Wf1T_d[i * QD : (i + 1) * QD, :])
                nc.gpsimd.dma_start(Wf2m[i][:], Wf2T_d[i * QD : (i + 1) * QD, :])

            repT = [pp.tile([n, S], F16, tag=f"repT{i}", name=f"repT{i}") for i, (o, n) in enumerate(DC)]
            repTm = [pp.tile([QD, S], F16, tag=f"rtm{i}", name=f"rtm{i}") for i in range(NQ)]
            rep_nat = pp.tile([128, 2 * D], F16)
            dep_nat = pp.tile([128, 2 * D], F16)
            head_nat = pp.tile([128, 2 * D], F16)
            E_t = pp.tile([128, 2 * D], F16)
            SERd = pp.tile([128, 2 * WB], F16)   # slot-major: col = c*WB + s*D + d
            SERn = pp.tile([128, 2 * WB], F16)
            bblk = pp.tile([128, 2 * WB], F16)   # d-major: col = c*WB + d*NS + s
            scd = [pp.tile([128, WB], F16, tag=f"scd{i}", name=f"scd{i}") for i in range(2)]
            scn = [pp.tile([128, WB], F16, tag=f"scn{i}", name=f"scn{i}") for i in range(2)]
            attn_nat = pp.tile([128, 2 * D], F16)
            a75 = [pp.tile([QD, S], F16, tag=f"a75_{i}", name=f"a75_{i}") for i in range(NQ)]
            th16 = [pp.tile([QD, S], F16, tag=f"th{i}", name=f"th{i}") for i in range(NQ)]
            Mb = pp.tile([QD, S], F16)

            # ---------- phase A ----------
            with (
                tc.tile_pool(name="pa_ps", bufs=2, space="PSUM") as pa_ps,
                tc.tile_pool(name="pa_sb", bufs=2) as pa_sb,
            ):
                def elu_from_psum(ps_ap, out_ap, n, bias=None):
                    # elu(x) = min(exp(x) - 1, relu(x)); optional per-partition
                    # bias column folded into both branches.
                    w = ps_ap.shape[1]
                    ex = pa_sb.tile([n, w], F16, tag="elu_e", name="elu_e")
                    nc.scalar.activation(
                        ex[:], ps_ap, AF.Exp,
                        bias=(bias if bias is not None else 0.0),
                    )
                    rl = pa_sb.tile([n, w], F16, tag="elu_r", name="elu_r")
                    nc.scalar.activation(
                        rl[:], ps_ap, AF.Relu,
                        bias=(bias if bias is not None else 0.0),
                    )
                    nc.vector.scalar_tensor_tensor(
                        out=out_ap, in0=ex[:], scalar=-1.0, in1=rl[:],
                        op0=OP.add, op1=OP.min,
                    )

                for i, (o, n) in enumerate(DC):
                    ps = pa_ps.tile([n, S], F32, tag="paT", name="paT")
                    for k in range(3):
                        nc.tensor.matmul(
                            ps[:], WfcT[k][:, o : o + n], inT[k][:],
                            start=(k == 0), stop=(k == 2),
                        )
                    elu_from_psum(ps[:], repT[i][:], n, bias=bcol[i][:])

                # Mb = broadcast of 0.5*rep_mask row (PE outer product)
                mps = pa_ps.tile([QD, S], F32, tag="mps", name="mps")
                nc.tensor.matmul(
                    mps[:], crow[0:1, O_ONES : O_ONES + QD],
                    crow[0:1, O_HM : O_HM + S],
                    start=True, stop=True,
                )
                nc.scalar.activation(Mb[:], mps[:], AF.Copy)

                # repTm copies (sbuf->sbuf DMA, 75-row quarters of repT)
                nc.sync.dma_start(repTm[0][:], repT[0][0:QD, :])
                nc.sync.dma_start(repTm[1][0:53, :], repT[0][QD:128, :])
                nc.scalar.dma_start(repTm[1][53:QD, :], repT[1][0:22, :])
                nc.scalar.dma_start(repTm[2][:], repT[1][22:97, :])
                nc.sync.dma_start(repTm[3][0:31, :], repT[1][97:128, :])
                nc.scalar.dma_start(repTm[3][31:QD, :], repT[2][0:44, :])

                for cc in range(2):
                    so = 128 * cc
                    ps = pa_ps.tile([128, D], F32, tag="paN", name="paN")
                    for k in range(3):
                        nc.tensor.matmul(
                            ps[:], repT[k][:, so : so + 128], W1T[k][:],
                            start=(k == 0), stop=False,
                        )
                    nc.tensor.matmul(
                        ps[:], crow[0:1, O_ONES : O_ONES + 128],
                        crow[0:1, O_B1 : O_B1 + D],
                        start=False, stop=True,
                    )
                    # E = exp(dep + maskbias) straight from psum; dep copy on ACT
                    nc.scalar.activation(
                        E_t[:, cc * D : (cc + 1) * D], ps[:], AF.Exp,
                        bias=maskb[:, cc : cc + 1], scale=1.0,
                    )
                    nc.scalar.activation(
                        dep_nat[:, cc * D : (cc + 1) * D], ps[:], AF.Copy
                    )

                    ps2 = pa_ps.tile([128, D], F32, tag="paN", name="paN")
                    for k in range(3):
                        nc.tensor.matmul(
                            ps2[:], repT[k][:, so : so + 128], W2T[k][:],
                            start=(k == 0), stop=(k == 2),
                        )
                    nc.scalar.activation(
                        head_nat[:, cc * D : (cc + 1) * D], ps2[:], AF.Copy
                    )

                # rep_nat last: only needed for the (late) num-series tt
                for cc in range(2):
                    so = 128 * cc
                    ps = pa_ps.tile([128, D], F32, tag="paN", name="paN")
                    for k in range(3):
                        nc.tensor.matmul(
                            ps[:], inT[k][:, so : so + 128], WfcT[k][:],
                            start=(k == 0), stop=False,
                        )
                    # b_fc bias runs along the free (d) dim here: rank-1 add
                    nc.tensor.matmul(
                        ps[:], crow[0:1, O_ONES : O_ONES + 128],
                        crow[0:1, O_BFC : O_BFC + D],
                        start=False, stop=True,
                    )
                    elu_from_psum(ps[:], rep_nat[:, cc * D : (cc + 1) * D], 128)

            # ---------- phase B + C ----------
            with (
                tc.tile_pool(name="pb_sb", bufs=2) as pb_sb,
                tc.tile_pool(name="pc_sb", bufs=2) as pc_sb,
            ):
                # PSUM era 1: suffix double-buffering (4 banks) + a single
                # rotating bank for the Wf1 gate partials (sequential
                # accumulation groups — concurrent groups in one tile are
                # illegal), saved to SBUF f16 and re-injected in era 2.
                _ps1 = [
                    tc.tile_pool(name="pb_ps", bufs=2, space="PSUM"),
                    tc.tile_pool(name="pcw_ps", bufs=1, space="PSUM"),
                ]
                pb_ps, pcw_ps = (c.__enter__() for c in _ps1)

                wf1sb = [pc_sb.tile([QD, S], F16, tag=f"w1s{i}", name=f"w1s{i}") for i in range(NQ)]
                for i, (go, gn) in enumerate(GC):
                    wfp = pcw_ps.tile([QD, S], F32, tag="wfp", name="wfp")
                    for q in range(NQ):
                        nc.tensor.matmul(
                            wfp[:], Wf1m[q][:, go : go + gn], repTm[q][:],
                            start=(q == 0), stop=False,
                        )
                    nc.tensor.matmul(
                        wfp[:], crow[0:1, O_BF + go : O_BF + go + gn],
                        crow[0:1, O_ONES : O_ONES + S],
                        start=False, stop=True,
                    )
                    nc.scalar.activation(wf1sb[i][:], wfp[:], AF.Copy)

                # Horner ladders -> slot-major series (DVE).  den finals
                # first; num series = den series * rep (2x-mode tt).
                sd4 = SERd[:].rearrange("p (c s d) -> p c s d", c=2, s=NS)
                sn4 = SERn[:].rearrange("p (c s d) -> p c s d", c=2, s=NS)
                E3 = E_t[:].rearrange("p (c d) -> p c d", c=2)
                rep3 = rep_nat[:].rearrange("p (c d) -> p c d", c=2)

                for m in range(K + 1):
                    n = K - m
                    s = K - m
                    if n == 0:
                        nc.vector.tensor_scalar(
                            out=sd4[:, :, s, :], in0=E3,
                            scalar1=QC[m][0], scalar2=None, op0=OP.mult,
                        )
                        continue
                    acc = pb_sb.tile([128, 2 * D], F16, tag=f"acc{m}", name=f"acc{m}")
                    nc.vector.tensor_scalar(
                        out=acc[:], in0=dep_nat[:],
                        scalar1=QC[m][n], scalar2=None, op0=OP.mult,
                    )
                    for j in range(n - 1, 0, -1):
                        nc.vector.scalar_tensor_tensor(
                            out=acc[:], in0=acc[:], scalar=QC[m][j], in1=dep_nat[:],
                            op0=OP.add, op1=OP.mult,
                        )
                    acc3 = acc[:].rearrange("p (c d) -> p c d", c=2)
                    nc.vector.scalar_tensor_tensor(
                        out=sd4[:, :, s, :], in0=acc3, scalar=QC[m][0],
                        in1=E3, op0=OP.add, op1=OP.mult,
                    )
                for m in range(K + 1):
                    s = K - m
                    nc.vector.tensor_tensor(
                        out=sn4[:, :, s, :], in0=sd4[:, :, s, :], in1=rep3,
                        op=OP.mult,
                    )

                # b-repeat tile AFTER the ladders: its strided DVE writes
                # overlap the den suffix matmuls on the PE instead of
                # delaying the series (only scans consume it)
                bb4 = bblk[:].rearrange("p (c d s) -> p c d s", c=2, s=NS)
                h3 = head_nat[:].rearrange("p (c d) -> p c d", c=2).unsqueeze(3)
                for s in range(1, NS):
                    nc.vector.tensor_scalar(
                        out=bb4[:, :, :, s : s + 1], in0=h3,
                        scalar1=0.0, scalar2=None, op0=OP.add,
                    )
                nc.vector.memset(bb4[:, :, :, 0:1], 0.0)

                def mv_slice(ser, cc, dlo, dn):
                    # moving AP iterating (d outer, s inner) over slot-major ser
                    v = ser[:].rearrange("p (c s d) -> p c d s", c=2, s=NS)
                    return v[:, cc, dlo : dlo + dn, :]

                # suffix matmuls + scans, kind-outer so all den matmuls run
                # during the bblk build and PE stays continuously busy
                for kind in range(2):
                    ser = SERd if kind == 0 else SERn
                    outs = scd if kind == 0 else scn
                    for q in range(NQ):
                        dlo = QD * q
                        p0 = pb_ps.tile([128, QW], F32, tag="sx0", name="sx0")
                        p1 = pb_ps.tile([128, QW], F32, tag="sx1", name="sx1")
                        nc.tensor.matmul(
                            p0[:], mats[:, 128:256], mv_slice(ser, 0, dlo, QD),
                            start=True, stop=False,
                        )
                        nc.tensor.matmul(
                            p0[:], mats[:, 256:384], mv_slice(ser, 1, dlo, QD),
                            start=False, stop=True,
                        )
                        nc.tensor.matmul(
                            p1[:], mats[:, 128:256], mv_slice(ser, 1, dlo, QD),
                            start=True, stop=True,
                        )
                        for cc in range(2):
                            nc.vector.tensor_tensor_scan(
                                out=outs[cc][:, q * QW : (q + 1) * QW],
                                data0=bblk[:, cc * WB + q * QW : cc * WB + (q + 1) * QW],
                                data1=(p0 if cc == 0 else p1)[:],
                                initial=0.0, op0=OP.mult, op1=OP.add,
                            )

                # PSUM era 2: transposes + gate accumulation (4 separate
                # one-bank tiles, concurrent groups in separate tiles are OK)
                for c in reversed(_ps1):
                    c.__exit__(None, None, None)
                _ps2 = [
                    tc.tile_pool(name="tp_ps", bufs=1, space="PSUM"),
                    tc.tile_pool(name="pg_ps", bufs=1, space="PSUM"),
                ]
                tp_ps, pg_ps = (c.__enter__() for c in _ps2)

                pcp = [pg_ps.tile([QD, S], F32, tag=f"pcp{i}", name=f"pcp{i}") for i in range(NQ)]

                def pcp_sl(i):
                    return pcp[i][:]

                # re-inject the saved Wf1 partials via identity matmuls
                for i in range(NQ):
                    nc.tensor.matmul(
                        pcp_sl(i), mats[0:QD, 0:QD], wf1sb[i][:],
                        start=True, stop=False,
                    )

                # divides + transposes per quarter
                for q in range(NQ):
                    dlo = QD * q
                    for cc in range(2):
                        dv = scd[cc][:, q * QW : (q + 1) * QW].rearrange(
                            "p (d s) -> p d s", s=NS)[:, :, K : K + 1]
                        nv = scn[cc][:, q * QW : (q + 1) * QW].rearrange(
                            "p (d s) -> p d s", s=NS)[:, :, K : K + 1]
                        den0 = pb_sb.tile([128, QD], F32, tag="den0", name="den0")
                        nc.vector.scalar_tensor_tensor(
                            out=den0[:].unsqueeze(2), in0=dv, scalar=0.0, in1=dv,
                            op0=OP.is_equal, op1=OP.add,
                        )
                        rcp = pb_sb.tile([128, QD], F32, tag="rcp", name="rcp")
                        nc.vector.reciprocal(out=rcp[:], in_=den0[:])
                        nc.vector.tensor_tensor(
                            out=attn_nat[:, cc * D + dlo : cc * D + dlo + QD].unsqueeze(2),
                            in0=nv, in1=rcp[:].unsqueeze(2), op=OP.mult,
                        )
                    for cc in range(2):
                        t75 = tp_ps.tile([QD, 128], F16, tag="t75", name="t75")
                        nc.tensor.transpose(
                            t75[:], attn_nat[:, cc * D + dlo : cc * D + dlo + QD],
                            mats[:, 0:128],
                        )
                        nc.scalar.activation(
                            a75[q][:, cc * 128 : (cc + 1) * 128], t75[:], AF.Copy
                        )
                    # Wf2 gate accumulation interleaved per quarter
                    for i, (go, gn) in enumerate(GC):
                        nc.tensor.matmul(
                            pcp_sl(i), Wf2m[q][:, go : go + gn], a75[q][:],
                            start=False, stop=(q == NQ - 1),
                        )

                for i in range(NQ):
                    nc.scalar.activation(th16[i][:], pcp_sl(i), AF.Tanh, scale=0.5)
                for c in reversed(_ps2):
                    c.__exit__(None, None, None)

                for q in range(NQ):
                    diff = pc_sb.tile([QD, S], F16, tag="diff", name="diff")
                    nc.vector.tensor_tensor(
                        out=diff[:], in0=repTm[q][:], in1=a75[q][:], op=OP.subtract
                    )
                    summ = pc_sb.tile([QD, S], F16, tag="summ", name="summ")
                    nc.vector.tensor_tensor(
                        out=summ[:], in0=repTm[q][:], in1=a75[q][:], op=OP.add
                    )
                    nc.vector.tensor_tensor(
                        out=diff[:], in0=th16[q][:], in1=diff[:], op=OP.mult
                    )
                    nc.vector.tensor_tensor(
                        out=summ[:], in0=summ[:], in1=diff[:], op=OP.add
                    )
                    outt = pc_sb.tile([QD, S], F32, tag="outt", name="outt")
                    nc.vector.tensor_tensor(
                        out=outt[:], in0=summ[:], in1=Mb[:], op=OP.mult
                    )
                    eng_o = nc.scalar if q % 2 == 1 else nc.sync
                    eng_o.dma_start(outT_d[q * QD : (q + 1) * QD, :], outt[:])

    nc.compile()
    return nc


def _host_prep(inputs, rep_mask, W_fc, b_fc, W1, W2, b1, W_f1, W_f2, b_f):
    f = np.float32
    h = np.float16
    su = (np.arange(128)[:, None] > np.arange(128)[None, :]).astype(h)
    mats = np.concatenate(
        [np.eye(128, dtype=h), su, np.ones((128, 128), dtype=h)], axis=1
    )
    in_maps = []
    for c in range(8):
        b = c // 2
        rm = rep_mask[b].astype(f)
        maskbias = np.stack(
            [(rm[0:128] - 1.0) * 30000.0, (rm[128:256] - 1.0) * 30000.0], axis=1
        ).astype(f)
        crow = np.zeros(CROW_W, dtype=h)
        crow[O_ONES : O_ONES + S] = 1.0
        crow[O_BFC : O_BFC + D] = b_fc.astype(h)
        crow[O_B1 : O_B1 + D] = b1.astype(h)
        crow[O_BF : O_BF + D] = b_f.astype(h)
        crow[O_HM : O_HM + S] = (0.5 * rm).astype(h)
        mp = {
            "inputsT": np.ascontiguousarray(inputs[b].T).astype(h),
            "W_fcT": np.ascontiguousarray(W_fc.T).astype(h),
            "W1T": np.ascontiguousarray(W1.T).astype(h),
            "W2T": np.ascontiguousarray(W2.T).astype(h),
            "Wf1T": np.ascontiguousarray(W_f1.T).astype(h),
            "Wf2T": np.ascontiguousarray(W_f2.T).astype(h),
            "consts_row": crow.reshape(1, CROW_W),
            "mats": mats,
            "maskbias": maskbias,
            "bfc_col": b_fc.astype(f).reshape(D, 1),
        }
        in_maps.append(mp)
    return in_maps


def _assemble(results):
    out = np.empty((B, S, D), dtype=np.float32)
    for b in range(B):
        out[b] = results[2 * b]["outT"].T
    return out


def kernel(**inputs):
    from concourse.bass_utils import run_bass_kernel_spmd

    if "nc" not in _CACHE:
        _CACHE["nc"] = _build_nc()
    nc = _CACHE["nc"]

    in_maps = _host_prep(**inputs)
    res = run_bass_kernel_spmd(nc, in_maps, list(range(8)))
    return _assemble(res.results)


# revision 52
# speedup vs baseline: 1.2279x; 1.0063x over previous
"""DiSA (directional self-attention) Bass kernel for Trainium2, 8 cores.

Factorized algorithm (no [S,S,D] intermediate): with
  w = e^{a+b} * psi(a+b),  psi(x) = exp(C*tanh(x/C) - x),  a=dep, b=head,
approximate psi by a degree-K polynomial P (weighted LS fit on the actual
a+b range).  Taylor expansion P(a+b) = sum_m b^m Q_m(a) makes the softmax
separable; e^b cancels in the ratio:
  attn_res[i,d] = sum_m b_i^m N_m(i) / sum_m b_i^m D_m(i)
  D_m(i) = suffix_{j>i}[ mask_j e^{a_j} Q_m(a_j) ]          (x rep_j for N_m)

Sharding: SOLO per batch — core c computes batch c//2 fully (pairs
duplicate work).  No collectives at all: on this runtime the first
collective pays a ~46us channel-init floor plus ~10us peer-start stagger,
which dwarfs the duplicated compute.  Host reads cores 0,2,4,6.

Mapping per core (full d=300):
  - series: DVE Horner ladders (fp16, flat [128,600] = 256 s x 300 d),
    mask folded into e^a via ACT bias; num series = den series * rep (2x tt).
  - suffix sums over j: PE matmuls, strict-upper-tri / all-ones
    stationaries; moving AP (d outer, s inner) -> d-major PSUM, K=4 ->
    [128,375] per 75-d quarter = one PSUM bank.
  - sum_m b^m X_m: DVE tensor_tensor_scan (state = b*state + X_m) over
    d-major PSUM with a b-repeat tile (0 at slot 0 of each d-block).
  - gate: pre-act accumulated in PSUM (Wf1 part early, Wf2 x attn late),
    tanh-sigmoid blend in T layout, outT [300,256] f32.
"""

import numpy as np
from math import comb as _comb

B, S, D = 4, 256, 300
C = 5.0
K = 4                  # psi polynomial degree
NS = K + 1             # slots per d-plane
WB = D * NS            # 1500 cols per c-block (slot-major series)
QD = 75                # d-planes per suffix/scan quarter
QW = QD * NS           # 375 cols per suffix psum (single PSUM bank)
NQ = D // QD           # 4 quarters

# crow packed consts: [ones(256) | b_fc(300) | b1(300) | b_f(300) | 0.5*rm(256)]
O_ONES, O_BFC, O_B1, O_BF, O_HM = 0, 256, 556, 856, 1156
CROW_W = 1412

_CACHE: dict = {}


def _q_coeffs():
    # weighted LS fit of psi on the actual a+b range (Gaussian weight
    # matching the data distribution; measured range ~[-5.6, 4.9])
    lo, hi, sig = -5.8, 5.1, 2.0
    xs = np.linspace(lo, hi, 12001)
    psi = np.exp(C * np.tanh(xs / C) - xs)
    w = np.exp(-xs ** 2 / (2 * sig ** 2)) + 1e-3
    V = np.vander(xs, K + 1, increasing=True) * w[:, None]
    c, *_ = np.linalg.lstsq(V, psi * w, rcond=None)
    return [[float(c[m + j] * _comb(m + j, m)) for j in range(K - m + 1)]
            for m in range(K + 1)]


QC = _q_coeffs()


def _chunks(total, step=128):
    return [(s, min(step, total - s)) for s in range(0, total, step)]


def _build_nc():
    import concourse.bass as bass
    import concourse.tile as tile
    from concourse import bacc, mybir

    F32 = mybir.dt.float32
    F16 = mybir.dt.float16
    AF = mybir.ActivationFunctionType
    OP = mybir.AluOpType

    nc = bacc.Bacc("TRN2", target_bir_lowering=False, debug=False, num_devices=8)

    def din(name, shape, dt=F16):
        return nc.dram_tensor(name, shape, dt, kind="ExternalInput").ap()

    inputsT_d = din("inputsT", [D, S])
    W_fcT_d = din("W_fcT", [D, D])
    W1T_d = din("W1T", [D, D])
    W2T_d = din("W2T", [D, D])
    Wf1T_d = din("Wf1T", [D, D])
    Wf2T_d = din("Wf2T", [D, D])
    crow_d = din("consts_row", [1, CROW_W])
    mats_d = din("mats", [128, 384])        # [ident | su_tri | ones]
    maskb_d = din("maskbias", [128, 2], F32)
    bcol_d = din("bfc_col", [D, 1], F32)    # b_fc per-partition column
    outT_d = nc.dram_tensor("outT", [D, S], F32, kind="ExternalOutput").ap()

    DC = _chunks(D)
    GC = [(QD * q, QD) for q in range(NQ)]  # 75-row g-quarters for gate psum

    with tile.TileContext(nc) as tc:
        with tc.tile_pool(name="persist", bufs=1) as pp:
            # ---- input DMAs: critical tensors first, spread across queues
            inT = [pp.tile([n, S], F16, tag=f"inT{i}", name=f"inT{i}") for i, (o, n) in enumerate(DC)]
            WfcT = [pp.tile([n, D], F16, tag=f"wfc{i}", name=f"wfc{i}") for i, (o, n) in enumerate(DC)]
            W1T = [pp.tile([n, D], F16, tag=f"w1{i}", name=f"w1_{i}") for i, (o, n) in enumerate(DC)]
            W2T = [pp.tile([n, D], F16, tag=f"w2{i}", name=f"w2_{i}") for i, (o, n) in enumerate(DC)]
            Wf1m = [pp.tile([QD, D], F16, tag=f"wg1{i}", name=f"wg1_{i}") for i in range(NQ)]
            Wf2m = [pp.tile([QD, D], F16, tag=f"wg2{i}", name=f"wg2_{i}") for i in range(NQ)]
            crow = pp.tile([1, CROW_W], F16)
            mats = pp.tile([128, 384], F16)
            maskb = pp.tile([128, 2], F32)
            bcol = [pp.tile([n, 1], F32, tag=f"bc{i}", name=f"bc{i}") for i, (o, n) in enumerate(DC)]

            for i, (o, n) in enumerate(DC):
                nc.sync.dma_start(inT[i][:], inputsT_d[o : o + n, :])
                nc.scalar.dma_start(WfcT[i][:], W_fcT_d[o : o + n, :])
            nc.sync.dma_start(crow[:], crow_d[:])
            nc.scalar.dma_start(maskb[:], maskb_d[:])
            for i, (o, n) in enumerate(DC):
                nc.sync.dma_start(W1T[i][:], W1T_d[o : o + n, :])
                nc.scalar.dma_start(W2T[i][:], W2T_d[o : o + n, :])
                nc.gpsimd.dma_start(bcol[i][:], bcol_d[o : o + n, :])
            nc.gpsimd.dma_start(mats[:], mats_d[:])
            for i in range(NQ):
                nc.gpsimd.dma_start(Wf1m[i][:], Wf1T_d[i * QD : (i + 1) * QD, :])
                nc.gpsimd.dma_start(Wf2m[i][:], Wf2T_d[i * QD : (i + 1) * QD, :])

            repT = [pp.tile([n, S], F16, tag=f"repT{i}", name=f"repT{i}") for i, (o, n) in enumerate(DC)]
            repTm = [pp.tile([QD, S], F16, tag=f"rtm{i}", name=f"rtm{i}") for i in range(NQ)]
            rep_nat = pp.tile([128, 2 * D], F16)
            dep_nat = pp.tile([128, 2 * D], F16)
            head_nat = pp.tile([128, 2 * D], F16)
            E_t = pp.tile([128, 2 * D], F16)
            SERd = pp.tile([128, 2 * WB], F16)   # slot-major: col = c*WB + s*D + d
            SERn = pp.tile([128, 2 * WB], F16)
            bblk = pp.tile([128, 2 * WB], F16)   # d-major: col = c*WB + d*NS + s
            scd = [pp.tile([128, WB], F16, tag=f"scd{i}", name=f"scd{i}") for i in range(2)]
            scn = [pp.tile([128, WB], F16, tag=f"scn{i}", name=f"scn{i}") for i in range(2)]
            attn_nat = pp.tile([128, 2 * D], F16)
            a75 = [pp.tile([QD, S], F16, tag=f"a75_{i}", name=f"a75_{i}") for i in range(NQ)]
            th16 = [pp.tile([QD, S], F16, tag=f"th{i}", name=f"th{i}") for i in range(NQ)]
            Mb = pp.tile([QD, S], F16)

            # ---------- phase A ----------
            with (
                tc.tile_pool(name="pa_ps", bufs=2, space="PSUM") as pa_ps,
                tc.tile_pool(name="pa_sb", bufs=2) as pa_sb,
            ):
                def elu_from_psum(ps_ap, out_ap, n, bias=None):
                    # elu(x) = min(exp(x) - 1, relu(x)); optional per-partition
                    # bias column folded into both branches.
                    w = ps_ap.shape[1]
                    ex = pa_sb.tile([n, w], F16, tag="elu_e", name="elu_e")
                    nc.scalar.activation(
                        ex[:], ps_ap, AF.Exp,
                        bias=(bias if bias is not None else 0.0),
                    )
                    rl = pa_sb.tile([n, w], F16, tag="elu_r", name="elu_r")
                    if bias is None:
                        nc.vector.tensor_scalar(
                            out=rl[:], in0=ps_ap, scalar1=0.0, scalar2=None, op0=OP.max
                        )
                    else:
                        nc.vector.tensor_scalar(
                            out=rl[:], in0=ps_ap, scalar1=bias, scalar2=0.0,
                            op0=OP.add, op1=OP.max,
                        )
                    nc.vector.scalar_tensor_tensor(
                        out=out_ap, in0=ex[:], scalar=-1.0, in1=rl[:],
                        op0=OP.add, op1=OP.min,
                    )

                for i, (o, n) in enumerate(DC):
                    ps = pa_ps.tile([n, S], F32, tag="paT", name="paT")
                    for k in range(3):
                        nc.tensor.matmul(
                            ps[:], WfcT[k][:, o : o + n], inT[k][:],
                            start=(k == 0), stop=(k == 2),
                        )
                    elu_from_psum(ps[:], repT[i][:], n, bias=bcol[i][:])

                # Mb = broadcast of 0.5*rep_mask row (PE outer product)
                mps = pa_ps.tile([QD, S], F32, tag="mps", name="mps")
                nc.tensor.matmul(
                    mps[:], crow[0:1, O_ONES : O_ONES + QD],
                    crow[0:1, O_HM : O_HM + S],
                    start=True, stop=True,
                )
                nc.scalar.activation(Mb[:], mps[:], AF.Copy)

                # repTm copies (sbuf->sbuf DMA, 75-row quarters of repT)
                nc.sync.dma_start(repTm[0][:], repT[0][0:QD, :])
                nc.sync.dma_start(repTm[1][0:53, :], repT[0][QD:128, :])
                nc.scalar.dma_start(repTm[1][53:QD, :], repT[1][0:22, :])
                nc.scalar.dma_start(repTm[2][:], repT[1][22:97, :])
                nc.sync.dma_start(repTm[3][0:31, :], repT[1][97:128, :])
                nc.scalar.dma_start(repTm[3][31:QD, :], repT[2][0:44, :])

                for cc in range(2):
                    so = 128 * cc
                    ps = pa_ps.tile([128, D], F32, tag="paN", name="paN")
                    for k in range(3):
                        nc.tensor.matmul(
                            ps[:], repT[k][:, so : so + 128], W1T[k][:],
                            start=(k == 0), stop=False,
                        )
                    nc.tensor.matmul(
                        ps[:], crow[0:1, O_ONES : O_ONES + 128],
                        crow[0:1, O_B1 : O_B1 + D],
                        start=False, stop=True,
                    )
                    # E = exp(dep + maskbias) straight from psum; dep copy on ACT
                    nc.scalar.activation(
                        E_t[:, cc * D : (cc + 1) * D], ps[:], AF.Exp,
                        bias=maskb[:, cc : cc + 1], scale=1.0,
                    )
                    nc.scalar.activation(
                        dep_nat[:, cc * D : (cc + 1) * D], ps[:], AF.Copy
                    )

                    ps2 = pa_ps.tile([128, D], F32, tag="paN", name="paN")
                    for k in range(3):
                        nc.tensor.matmul(
                            ps2[:], repT[k][:, so : so + 128], W2T[k][:],
                            start=(k == 0), stop=(k == 2),
                        )
                    nc.scalar.activation(
                        head_nat[:, cc * D : (cc + 1) * D], ps2[:], AF.Copy
                    )

                # rep_nat last: only needed for the (late) num-series tt
                for cc in range(2):
                    so = 128 * cc
                    ps = pa_ps.tile([128, D], F32, tag="paN", name="paN")
                    for k in range(3):
                        nc.tensor.matmul(
                            ps[:], inT[k][:, so : so + 128], WfcT[k][:],
                            start=(k == 0), stop=False,
                        )
                    # b_fc bias runs along the free (d) dim here: rank-1 add
                    nc.tensor.matmul(
                        ps[:], crow[0:1, O_ONES : O_ONES + 128],
                        crow[0:1, O_BFC : O_BFC + D],
                        start=False, stop=True,
                    )
                    elu_from_psum(ps[:], rep_nat[:, cc * D : (cc + 1) * D], 128)

            # ---------- phase B + C ----------
            with (
                tc.tile_pool(name="pb_sb", bufs=2) as pb_sb,
                tc.tile_pool(name="pc_sb", bufs=2) as pc_sb,
            ):
                # PSUM era 1: suffix double-buffering (4 banks) + a single
                # rotating bank for the Wf1 gate partials (sequential
                # accumulation groups — concurrent groups in one tile are
                # illegal), saved to SBUF f16 and re-injected in era 2.
                _ps1 = [
                    tc.tile_pool(name="pb_ps", bufs=2, space="PSUM"),
                    tc.tile_pool(name="pcw_ps", bufs=1, space="PSUM"),
                ]
                pb_ps, pcw_ps = (c.__enter__() for c in _ps1)

                wf1sb = [pc_sb.tile([QD, S], F16, tag=f"w1s{i}", name=f"w1s{i}") for i in range(NQ)]
                for i, (go, gn) in enumerate(GC):
                    wfp = pcw_ps.tile([QD, S], F32, tag="wfp", name="wfp")
                    for q in range(NQ):
                        nc.tensor.matmul(
                            wfp[:], Wf1m[q][:, go : go + gn], repTm[q][:],
                            start=(q == 0), stop=False,
                        )
                    nc.tensor.matmul(
                        wfp[:], crow[0:1, O_BF + go : O_BF + go + gn],
                        crow[0:1, O_ONES : O_ONES + S],
                        start=False, stop=True,
                    )
                    nc.scalar.activation(wf1sb[i][:], wfp[:], AF.Copy)

                # Horner ladders -> slot-major series (DVE).  den finals
                # first; num series = den series * rep (2x-mode tt).
                sd4 = SERd[:].rearrange("p (c s d) -> p c s d", c=2, s=NS)
                sn4 = SERn[:].rearrange("p (c s d) -> p c s d", c=2, s=NS)
                E3 = E_t[:].rearrange("p (c d) -> p c d", c=2)
                rep3 = rep_nat[:].rearrange("p (c d) -> p c d", c=2)

                for m in range(K + 1):
                    n = K - m
                    s = K - m
                    if n == 0:
                        nc.vector.tensor_scalar(
                            out=sd4[:, :, s, :], in0=E3,
                            scalar1=QC[m][0], scalar2=None, op0=OP.mult,
                        )
                        continue
                    acc = pb_sb.tile([128, 2 * D], F16, tag=f"acc{m}", name=f"acc{m}")
                    nc.vector.tensor_scalar(
                        out=acc[:], in0=dep_nat[:],
                        scalar1=QC[m][n], scalar2=None, op0=OP.mult,
                    )
                    for j in range(n - 1, 0, -1):
                        nc.vector.scalar_tensor_tensor(
                            out=acc[:], in0=acc[:], scalar=QC[m][j], in1=dep_nat[:],
                            op0=OP.add, op1=OP.mult,
                        )
                    acc3 = acc[:].rearrange("p (c d) -> p c d", c=2)
                    nc.vector.scalar_tensor_tensor(
                        out=sd4[:, :, s, :], in0=acc3, scalar=QC[m][0],
                        in1=E3, op0=OP.add, op1=OP.mult,
                    )
                for m in range(K + 1):
                    s = K - m
                    nc.vector.tensor_tensor(
                        out=sn4[:, :, s, :], in0=sd4[:, :, s, :], in1=rep3,
                        op=OP.mult,
                    )

                # b-repeat tile AFTER the ladders: its strided DVE writes
                # overlap the den suffix matmuls on the PE instead of
                # delaying the series (only scans consume it)
                bb4 = bblk[:].rearrange("p (c d s) -> p c d s", c=2, s=NS)
                h3 = head_nat[:].rearrange("p (c d) -> p c d", c=2).unsqueeze(3)
                for s in range(1, NS):
                    nc.vector.tensor_scalar(
                        out=bb4[:, :, :, s : s + 1], in0=h3,
                        scalar1=0.0, scalar2=None, op0=OP.add,
                    )
                nc.vector.memset(bb4[:, :, :, 0:1], 0.0)

                def mv_slice(ser, cc, dlo, dn):
                    # moving AP iterating (d outer, s inner) over slot-major ser
                    v = ser[:].rearrange("p (c s d) -> p c d s", c=2, s=NS)
                    return v[:, cc, dlo : dlo + dn, :]

                # suffix matmuls + scans, kind-outer so all den matmuls run
                # during the bblk build and PE stays continuously busy
                for kind in range(2):
                    ser = SERd if kind == 0 else SERn
                    outs = scd if kind == 0 else scn
                    for q in range(NQ):
                        dlo = QD * q
                        p0 = pb_ps.tile([128, QW], F32, tag="sx0", name="sx0")
                        p1 = pb_ps.tile([128, QW], F32, tag="sx1", name="sx1")
                        nc.tensor.matmul(
                            p0[:], mats[:, 128:256], mv_slice(ser, 0, dlo, QD),
                            start=True, stop=False,
                        )
                        nc.tensor.matmul(
                            p0[:], mats[:, 256:384], mv_slice(ser, 1, dlo, QD),
                            start=False, stop=True,
                        )
                        nc.tensor.matmul(
                            p1[:], mats[:, 128:256], mv_slice(ser, 1, dlo, QD),
                            start=True, stop=True,
                        )
                        for cc in range(2):
                            nc.vector.tensor_tensor_scan(
                                out=outs[cc][:, q * QW : (q + 1) * QW],
                                data0=bblk[:, cc * WB + q * QW : cc * WB + (q + 1) * QW],
                                data1=(p0 if cc == 0 else p1)[:],
                                initial=0.0, op0=OP.mult, op1=OP.add,
                            )

                # PSUM era 2: transposes + gate accumulation (4 separate
                # one-bank tiles, concurrent groups in separate tiles are OK)
                for c in reversed(_ps1):
                    c.__exit__(None, None, None)
                _ps2 = [
                    tc.tile_pool(name="tp_ps", bufs=1, space="PSUM"),
                    tc.tile_pool(name="pg_ps", bufs=1, space="PSUM"),
                ]
                tp_ps, pg_ps = (c.__enter__() for c in _ps2)

                pcp = [pg_ps.tile([QD, S], F32, tag=f"pcp{i}", name=f"pcp{i}") for i in range(NQ)]

                def pcp_sl(i):
                    return pcp[i][:]

                # re-inject the saved Wf1 partials via identity matmuls
                for i in range(NQ):
                    nc.tensor.matmul(
                        pcp_sl(i), mats[0:QD, 0:QD], wf1sb[i][:],
                        start=True, stop=False,
                    )

                # divides + transposes per quarter
                for q in range(NQ):
                    dlo = QD * q
                    for cc in range(2):
                        dv = scd[cc][:, q * QW : (q + 1) * QW].rearrange(
                            "p (d s) -> p d s", s=NS)[:, :, K : K + 1]
                        nv = scn[cc][:, q * QW : (q + 1) * QW].rearrange(
                            "p (d s) -> p d s", s=NS)[:, :, K : K + 1]
                        den0 = pb_sb.tile([128, QD], F32, tag="den0", name="den0")
                        nc.vector.scalar_tensor_tensor(
                            out=den0[:].unsqueeze(2), in0=dv, scalar=0.0, in1=dv,
                            op0=OP.is_equal, op1=OP.add,
                        )
                        rcp = pb_sb.tile([128, QD], F32, tag="rcp", name="rcp")
                        nc.vector.reciprocal(out=rcp[:], in_=den0[:])
                        nc.vector.tensor_tensor(
                            out=attn_nat[:, cc * D + dlo : cc * D + dlo + QD].unsqueeze(2),
                            in0=nv, in1=rcp[:].unsqueeze(2), op=OP.mult,
                        )
                    for cc in range(2):
                        t75 = tp_ps.tile([QD, 128], F16, tag="t75", name="t75")
                        nc.tensor.transpose(
                            t75[:], attn_nat[:, cc * D + dlo : cc * D + dlo + QD],
                            mats[:, 0:128],
                        )
                        nc.scalar.activation(
                            a75[q][:, cc * 128 : (cc + 1) * 128], t75[:], AF.Copy
                        )
                    # Wf2 gate accumulation interleaved per quarter
                    for i, (go, gn) in enumerate(GC):
                        nc.tensor.matmul(
                            pcp_sl(i), Wf2m[q][:, go : go + gn], a75[q][:],
                            start=False, stop=(q == NQ - 1),
                        )

                for i in range(NQ):
                    nc.scalar.activation(th16[i][:], pcp_sl(i), AF.Tanh, scale=0.5)
                for c in reversed(_ps2):
                    c.__exit__(None, None, None)

                for q in range(NQ):
                    diff = pc_sb.tile([QD, S], F16, tag="diff", name="diff")
                    nc.vector.tensor_tensor(
                        out=diff[:], in0=repTm[q][:], in1=a75[q][:], op=OP.subtract
                    )
                    summ = pc_sb.tile([QD, S], F16, tag="summ", name="summ")
                    nc.vector.tensor_tensor(
                        out=summ[:], in0=repTm[q][:], in1=a75[q][:], op=OP.add
                    )
                    nc.vector.tensor_tensor(
                        out=diff[:], in0=th16[q][:], in1=diff[:], op=OP.mult
                    )
                    nc.vector.tensor_tensor(
                        out=summ[:], in0=summ[:], in1=diff[:], op=OP.add
                    )
                    outt = pc_sb.tile([QD, S], F32, tag="outt", name="outt")
                    nc.vector.tensor_tensor(
                        out=outt[:], in0=summ[:], in1=Mb[:], op=OP.mult
                    )
                    eng_o = nc.scalar if q % 2 == 1 else nc.sync
                    eng_o.dma_start(outT_d[q * QD : (q + 1) * QD, :], outt[:])

    nc.compile()
    return nc


def _host_prep(inputs, rep_mask, W_fc, b_fc, W1, W2, b1, W_f1, W_f2, b_f):
    f = np.float32
    h = np.float16
    su = (np.arange(128)[:, None] > np.arange(128)[None, :]).astype(h)
    mats = np.concatenate(
        [np.eye(128, dtype=h), su, np.ones((128, 128), dtype=h)], axis=1
    )
    in_maps = []
    for c in range(8):
        b = c // 2
        rm = rep_mask[b].astype(f)
        maskbias = np.stack(
            [(rm[0:128] - 1.0) * 30000.0, (rm[128:256] - 1.0) * 30000.0], axis=1
        ).astype(f)
        crow = np.zeros(CROW_W, dtype=h)
        crow[O_ONES : O_ONES + S] = 1.0
        crow[O_BFC : O_BFC + D] = b_fc.astype(h)
        crow[O_B1 : O_B1 + D] = b1.astype(h)
        crow[O_BF : O_BF + D] = b_f.astype(h)
        crow[O_HM : O_HM + S] = (0.5 * rm).astype(h)
        mp = {
            "inputsT": np.ascontiguousarray(inputs[b].T).astype(h),
            "W_fcT": np.ascontiguousarray(W_fc.T).astype(h),
            "W1T": np.ascontiguousarray(W1.T).astype(h),
            "W2T": np.ascontiguousarray(W2.T).astype(h),
            "Wf1T": np.ascontiguousarray(W_f1.T).astype(h),
            "Wf2T": np.ascontiguousarray(W_f2.T).astype(h),
            "consts_row": crow.reshape(1, CROW_W),
            "mats": mats,
            "maskbias": maskbias,
            "bfc_col": b_fc.astype(f).reshape(D, 1),
        }
        in_maps.append(mp)
    return in_maps


def _assemble(results):
    out = np.empty((B, S, D), dtype=np.float32)
    for b in range(B):
        out[b] = results[2 * b]["outT"].T
    return out


def kernel(**inputs):
    from concourse.bass_utils import run_bass_kernel_spmd

    if "nc" not in _CACHE:
        _CACHE["nc"] = _build_nc()
    nc = _CACHE["nc"]

    in_maps = _host_prep(**inputs)
    res = run_bass_kernel_spmd(nc, in_maps, list(range(8)))
    return _assemble(res.results)
